# revision 30
# baseline (speedup 1.0000x reference)
"""Trainium2 Bass kernel for MultiHeadAttention + residual + LayerNorm.

Problem: B=2, S=2048, D=768, H=12 heads, dk=64.
  Q/K/V = X @ W; scores = Q K^T / 8; P = softmax(scores); ctx = P V;
  out = LayerNorm(ctx @ Wo + input_Q), LN weight=1 bias=0 eps=1e-5.

Sharding (8 cores, zero collectives): core c -> batch b=c//4, query rows
r=c%4 -> rows [r*512,(r+1)*512). Each core replicates the full K/V
projections for its batch (2048 keys, 12 heads) and computes attention +
out-proj + residual + LN for its 512 query rows. Host gathers 8x[512,768].

X inputs arrive pre-transposed from the host (X^T layouts), so the device
issues only plain partition-major DMAs -- no descriptor-based
DmaTransposeAnt (whose completion signalling raced with consumers and made
the previous revision nondeterministic on HW).

Schedule: engines execute queues in emission order, so emission is the
schedule. Phase A does Q-proj, K-proj m-chunk 0, and all of V-proj.
The remaining 20 K-proj (m,n) chunks are interleaved into the attention
head loop (2 per head) to fill PE stalls while ACT runs the exps: head
pair m-1 computes kT[m], just in time for heads 2m/2m+1. Scores are
computed transposed S^T [keys, q] in waves of 2 key tiles; softmax skips
the max subtraction (|s| < ~11); denominators come free via a
ones-augmented 65th column in V; normalization uses a K=1 broadcast
matmul of the reciprocal denominators. PSUM: s_ps 2x2 + ctx 2 + kproj 1
+ rowbcast 1 = 8 banks.

get_program(loop_n=N) wraps the identical body in a tc.For_i hardware
loop (full all-engine barrier at the back edge) -- used by test.py to
measure per-invocation device time with the ~70ms axon dispatch overhead
amortized away.
"""

import sys

if "/opt/trn_rl_repo" not in sys.path:
    sys.path.insert(0, "/opt/trn_rl_repo")

import numpy as np
from ml_dtypes import bfloat16

B = 2
S = 2048
D = 768
H = 12
DK = 64
QR = 512          # query rows per core
NCORES = 8
KT = 128          # key tile (keys per score matmul)
NKT = S // KT     # 16 key tiles

_PROGRAMS = {}


def _build_program(loop_n=None, unroll=1):
    from concourse import bacc, mybir, tile

    dt = mybir.dt
    f32 = dt.float32
    bf16 = dt.bfloat16
    f32r = dt.float32r
    Exp = mybir.ActivationFunctionType.Exp
    Sqrt = mybir.ActivationFunctionType.Sqrt
    Copy = mybir.ActivationFunctionType.Copy
    Square = mybir.ActivationFunctionType.Square
    Alu = mybir.AluOpType

    nc = bacc.Bacc(
        "TRN2",
        target_bir_lowering=False,
        debug=False,
        enable_asserts=False,
        num_devices=NCORES,
    )

    # All inputs arrive pre-arranged by the host into the exact SBUF layout
    # [128 partitions, chunk, n] so every DMA is a single fully-contiguous
    # transfer (per-partition runs of 3-9KB).
    xq_bf = nc.dram_tensor("xq_bf", [128, 4, D], bf16,
                           kind="ExternalInput").ap()
    xqT_d = nc.dram_tensor("xqT_bf", [128, 6, QR], bf16,
                           kind="ExternalInput").ap()
    xkT_d = nc.dram_tensor("xkT_bf", [128, 6, S], bf16,
                           kind="ExternalInput").ap()
    xvT_d = nc.dram_tensor("xvT_bf", [128, 6, S], bf16,
                           kind="ExternalInput").ap()
    wq_d = nc.dram_tensor("wq_bf", [128, 6, D], bf16,
                          kind="ExternalInput").ap()
    wk_d = nc.dram_tensor("wk_bf", [128, 6, D], bf16,
                          kind="ExternalInput").ap()
    wv_d = nc.dram_tensor("wv_bf", [128, 6, D], bf16,
                          kind="ExternalInput").ap()
    wo_d = nc.dram_tensor("wo", [128, 6, D], bf16, kind="ExternalInput").ap()
    out_d = nc.dram_tensor("out", [QR, D], f32, kind="ExternalOutput").ap()

    def emit(tc, pers):
        def pt(shape, dtype, tag):
            return pers.tile(shape, dtype, name=tag, tag=tag)

        ones64f = pt([1, DK], f32, "ones64f")
        nc.vector.memset(ones64f[:], 1.0)
        ones64 = pt([1, DK], f32r, "ones64")
        with nc.allow_low_precision(reason="f32r == fp32 bits"):
            nc.vector.tensor_copy(out=ones64[:], in_=ones64f[:])
        eps_t = pt([128, 1], f32, "eps_t")
        nc.vector.memset(eps_t[:], 1e-5)
        # shift exp input from [-10.1, 10.1] to [-20.2, 0]: softmax is
        # shift-invariant, but the HW Exp table is accurate for x <= 0
        zbias = pt([128, 1], f32, "zbias")
        nc.vector.memset(zbias[:], -10.1)
        ident = pt([128, 128], bf16, "ident")
        nc.gpsimd.memset(ident[:], 0.0)
        nc.gpsimd.affine_select(
            out=ident[:], in_=ident[:],
            compare_op=Alu.not_equal, fill=1.0,
            base=0, pattern=[[-1, 128]], channel_multiplier=1,
        )

        wo_sb = pt([128, 6, D], bf16, "wo_sb")
        res_sb = pt([128, 4, D], bf16, "res_sb")

        qT = [pt([128, QR], bf16, f"qT{t}") for t in range(6)]
        kT = [pt([128, S], bf16, f"kT{t}") for t in range(6)]
        v_sb = [pt([128, H, DK + 1], bf16, f"v{k}") for k in range(NKT)]
        ctxT = [pt([128, QR], bf16, f"ctxT{t}") for t in range(6)]
        # xkT/wk stay live through attention (interleaved K-proj chunks)
        xkT = pt([128, 6, S], bf16, "xkT")
        wk_sb = pt([128, 6, D], bf16, "wk_sb")

        # ---- Phase A: Q proj, K proj m=0, V proj ----
        with tc.sbuf_pool(name="pa_sb", bufs=1) as pa, \
                tc.psum_pool(name="pa_ps", bufs=1) as pps:
            xqT = pa.tile([128, 6, QR], bf16, name="xqT", tag="xqT")
            xvT = pa.tile([128, 6, S], bf16, name="xvT", tag="xvT")
            wq_sb = pa.tile([128, 6, D], bf16, name="wq_sb", tag="wq_sb")
            wv_sb = pa.tile([128, 6, D], bf16, name="wv_sb", tag="wv_sb")

            # Everything arrives host-pre-arranged in SBUF layout, so each
            # DMA below is one fully-contiguous transfer (fixing the chunk
            # index still gives contiguous per-partition runs). Dispatch
            # spread across SP + ACT queues (the only HWDGE engines).
            # Order = consumption order: Q (fine chunks so the first
            # matmul starts after ~0.3MB), V, K. Per-chunk transfers also
            # give consumers chunk-granular dependencies instead of
            # whole-tensor ones.
            qdma = [nc.scalar, nc.sync]
            nc.sync.dma_start(out=wq_sb[:, 0:1, :], in_=wq_d[:, 0:1, :])
            nc.scalar.dma_start(out=xqT[:, 0:1, :], in_=xqT_d[:, 0:1, :])
            nc.sync.dma_start(out=wq_sb[:, 1:3, :], in_=wq_d[:, 1:3, :])
            nc.scalar.dma_start(out=xqT[:, 1:3, :], in_=xqT_d[:, 1:3, :])
            nc.sync.dma_start(out=wq_sb[:, 3:6, :], in_=wq_d[:, 3:6, :])
            nc.scalar.dma_start(out=xqT[:, 3:6, :], in_=xqT_d[:, 3:6, :])
            # interleave wv halves with xvT chunks so V-proj's first
            # matmuls (needing wv[:,0:3] + early xvT chunks) start sooner
            nc.sync.dma_start(out=wv_sb[:, 0:3, :], in_=wv_d[:, 0:3, :])
            for t in range(3):
                qdma[t % 2].dma_start(out=xvT[:, t, :], in_=xvT_d[:, t, :])
            nc.sync.dma_start(out=wv_sb[:, 3:6, :], in_=wv_d[:, 3:6, :])
            for t in range(3, 6):
                qdma[t % 2].dma_start(out=xvT[:, t, :], in_=xvT_d[:, t, :])
            nc.scalar.dma_start(out=wk_sb[:], in_=wk_d[:])
            for t in range(6):
                qdma[t % 2].dma_start(out=xkT[:, t, :], in_=xkT_d[:, t, :])

            # Q^T [768, 512] = Wq^T @ Xq^T, scale 1/8 folded into evacuation.
            # Two half-contraction passes so PE starts early (6 open PSUM
            # accumulators). Pass 1 is t-major: the first 6 matmuls need
            # only the t=0 slices of wq/xqT (~0.3MB of DMA).
            q_ps = [pps.tile([128, QR], f32, name=f"q_ps{m}", tag=f"q_ps{m}")
                    for m in range(6)]
            for t in range(3):
                for m in range(6):
                    nc.tensor.matmul(
                        q_ps[m][:],
                        lhsT=wq_sb[:, t, 128 * m:128 * (m + 1)],
                        rhs=xqT[:, t, :],
                        start=(t == 0), stop=False,
                    )
            for m in range(6):
                for t in range(3, 6):
                    nc.tensor.matmul(
                        q_ps[m][:],
                        lhsT=wq_sb[:, t, 128 * m:128 * (m + 1)],
                        rhs=xqT[:, t, :],
                        start=False, stop=(t == 5),
                    )
                nc.vector.tensor_scalar_mul(qT[m][:], q_ps[m][:], 0.125)

            # V [2048, 768] natural, stored per key-tile as [128, 12, 65]
            # (65th column = ones for softmax denominators)
            for k in range(NKT):
                nc.vector.memset(v_sb[k][:, :, DK:DK + 1], 1.0)
            for k in range(NKT):
                for half, (c0, c1) in enumerate(((0, 512), (512, 768))):
                    v_ps = pps.tile([128, 512], f32, name="v_ps", tag="v_ps",
                                    bufs=2)
                    vp = v_ps[:, 0:c1 - c0]
                    for t in range(6):
                        nc.tensor.matmul(
                            vp,
                            lhsT=xvT[:, t, KT * k:KT * (k + 1)],
                            rhs=wv_sb[:, t, c0:c1],
                            start=(t == 0), stop=(t == 5),
                        )
                    h0, h1 = c0 // DK, c1 // DK
                    nc.vector.tensor_copy(
                        out=v_sb[k][:, h0:h1, 0:DK],
                        in_=vp.rearrange("p (h d) -> p h d", d=DK),
                    )

            # K^T m-chunk 0 (rows 0:128 of K^T) — heads 0,1 need it first.
            # Evacuate on ACT (idle during phase A; Pool can't read PSUM)
            # so the copies don't queue behind the V evacuations on DVE
            # (in-order queues).
            for n in range(4):
                k_ps = pps.tile([128, 512], f32, name="k_ps", tag="v_ps",
                                bufs=2)
                for t in range(6):
                    nc.tensor.matmul(
                        k_ps[:],
                        lhsT=wk_sb[:, t, 0:128],
                        rhs=xkT[:, t, 512 * n:512 * (n + 1)],
                        start=(t == 0), stop=(t == 5),
                    )
                nc.scalar.activation(kT[0][:, 512 * n:512 * (n + 1)],
                                     k_ps[:], Copy)

        # ---- Phase B: attention, with K-proj chunks interleaved ----
        # head pair p computes kT[p+1] chunks (2 per head), used by heads
        # 2(p+1), 2(p+1)+1.
        kchunks = [(m, n) for m in range(1, 6) for n in range(4)]
        waves = [(2 * w, 2 * w + 1) for w in range(NKT // 2)]
        with tc.psum_pool(name="att_ps", bufs=1) as aps, \
                tc.sbuf_pool(name="att_sb", bufs=1) as asb:
            # stage phase-C operands during the attention window
            nc.sync.dma_start(out=wo_sb[:], in_=wo_d[:])
            nc.sync.dma_start(out=res_sb[:], in_=xq_bf[:])

            def emit_kchunk(m, n):
                k_ps = aps.tile([128, 512], f32, name="scratch", tag="scratch")
                for t2 in range(6):
                    nc.tensor.matmul(
                        k_ps[:],
                        lhsT=wk_sb[:, t2, 128 * m:128 * (m + 1)],
                        rhs=xkT[:, t2, 512 * n:512 * (n + 1)],
                        start=(t2 == 0), stop=(t2 == 5),
                    )
                nc.vector.tensor_copy(out=kT[m][:, 512 * n:512 * (n + 1)],
                                      in_=k_ps[:])

            # Deferred per-head normalization: the rb broadcast matmul
            # depends on the DVE reciprocal of the denominators, so emitting
            # it right after the head's last ctx matmul stalls PE on a
            # cross-engine handoff. Instead recip+cevac are issued
            # immediately (freeing the ctx PSUM bank) and the PE-side
            # finalize runs after the NEXT head's first score wave, by which
            # time the reciprocal has long completed.
            pending_fin = []

            def chunks_for(h):
                # heads 0..7: two chunks each (kT[1..4], just in time for
                # head pair t+1). kT[5]'s four chunks are spread over heads
                # 8,9,10 so the tail heads keep PE fill work: head 10's
                # chunk (5,3) covers keys 1536:2048, first read at its own
                # wave 6 -- emitted at wave 2, ready by wave 4.
                t = h // 2
                if t < 4:
                    return kchunks[4 * t + 2 * (h % 2):
                                   4 * t + 2 * (h % 2) + 2]
                if h == 8:
                    return kchunks[16:18]
                if h == 9:
                    return kchunks[18:19]
                if h == 10:
                    return kchunks[19:20]
                return []

            for h in range(H):
                t, po = h // 2, DK * (h % 2)
                ctx_ps = aps.tile([DK + 1, QR], f32, name="ctx_ps",
                                  tag="ctx_ps")
                mine = chunks_for(h)

                def score_wave(kts):
                    s_ps = aps.tile([128, 2, QR], f32, name="s_ps",
                                    tag="s_ps", bufs=3)
                    for i, k in enumerate(kts):
                        nc.tensor.matmul(
                            s_ps[:, i, :],
                            lhsT=kT[t][po:po + DK, KT * k:KT * (k + 1)],
                            rhs=qT[t][po:po + DK, :],
                            start=True, stop=True,
                        )
                    return s_ps

                def softmax_ctx(s_ps, kts):
                    pT = asb.tile([128, 2, QR], bf16, name="pT", tag="pT",
                                  bufs=3)
                    nc.scalar.activation(pT[:], s_ps[:], Exp, bias=zbias[:])
                    for i, k in enumerate(kts):
                        nc.tensor.matmul(
                            ctx_ps[:],
                            lhsT=v_sb[k][:, h, :],
                            rhs=pT[:, i, :],
                            start=(k == 0), stop=(k == NKT - 1),
                        )

                pend = []
                for w, kts in enumerate(waves):
                    s_ps = score_wave(kts)
                    if w == 0 and pending_fin:
                        pending_fin.pop(0)()
                    if w == 2 and len(mine) > 0:
                        emit_kchunk(*mine[0])
                    elif w == 5 and len(mine) > 1:
                        emit_kchunk(*mine[1])
                    pend.append((s_ps, kts))
                    if len(pend) > 2:
                        softmax_ctx(*pend.pop(0))
                for p in pend:
                    softmax_ctx(*p)

                # normalize: ctxT[head] = ctx * (1/denom), denom in row 64.
                # recip + cevac now; the PE rb matmul + final mul deferred.
                recip = asb.tile([1, QR], f32r, name="recip", tag="recip",
                                 bufs=2)
                with nc.allow_low_precision(reason="f32r == fp32 bits"):
                    nc.vector.reciprocal(recip[:], ctx_ps[DK:DK + 1, :])
                cevac = asb.tile([DK, QR], f32r, name="cevac", tag="cevac",
                                 bufs=2)
                nc.vector.tensor_copy(out=cevac[:], in_=ctx_ps[0:DK, :])

                def fin(t=t, po=po, recip=recip, cevac=cevac):
                    rb_t = aps.tile([128, QR], f32, name="scratch",
                                    tag="scratch")
                    rb_ps = rb_t[0:DK, :]
                    nc.tensor.matmul(rb_ps[:], lhsT=ones64[:], rhs=recip[:],
                                     start=True, stop=True)
                    with nc.allow_low_precision(
                            reason="ctx in bf16 for out-proj"):
                        nc.vector.tensor_mul(ctxT[t][po:po + DK, :],
                                             cevac[:], rb_ps[:])

                pending_fin.append(fin)

            for fin in pending_fin:
                fin()

        # ---- Phase C: out-projection + residual + LayerNorm ----
        # Residual is folded into the PSUM accumulation via an identity
        # matmul (lhsT=I128, rhs=res rows); LN mean/var come from ACT
        # Copy/Square with accum_out row-sums, keeping DVE off the
        # critical tail path.
        # Each chunk's LN finalize (everything after the Square) waits on a
        # DVE round trip; emitting it inline would head-of-line-block the
        # next chunk's PSUM-evacuating ACT copies in the in-order ACT
        # queue. Defer it until after the next chunk's ACT work is queued.
        with tc.psum_pool(name="out_ps", bufs=2) as ops, \
                tc.sbuf_pool(name="ln_sbp", bufs=2) as lnp:
            pending_ln = []
            for m in range(4):
                ms = slice(128 * m, 128 * (m + 1))
                o_ps = ops.tile([128, 512], f32, name="o_ps", tag="o_ps")
                o_ps2 = ops.tile([128, 256], f32, name="o_ps2", tag="o_ps2")
                for t in range(6):
                    nc.tensor.matmul(o_ps[:], lhsT=ctxT[t][:, ms],
                                     rhs=wo_sb[:, t, 0:512],
                                     start=(t == 0), stop=False)
                nc.tensor.matmul(o_ps[:], lhsT=ident[:],
                                 rhs=res_sb[:, m, 0:512],
                                 start=False, stop=True)
                for t in range(6):
                    nc.tensor.matmul(o_ps2[:], lhsT=ctxT[t][:, ms],
                                     rhs=wo_sb[:, t, 512:768],
                                     start=(t == 0), stop=False)
                nc.tensor.matmul(o_ps2[:], lhsT=ident[:],
                                 rhs=res_sb[:, m, 512:768],
                                 start=False, stop=True)

                ln_sb = lnp.tile([128, D], f32, name="ln_sb", tag="ln_sb")
                s1 = lnp.tile([128, 1], f32, name="s1", tag="s1")
                s2 = lnp.tile([128, 1], f32, name="s2", tag="s2")
                nc.scalar.activation(ln_sb[:, 0:512], o_ps[:], Copy,
                                     accum_out=s1[:])
                nc.scalar.activation(ln_sb[:, 512:768], o_ps2[:], Copy,
                                     accum_out=s2[:])
                sqd = lnp.tile([128, D], f32, name="sqd", tag="sqd")
                ssq = lnp.tile([128, 1], f32, name="ssq", tag="ssq")
                nc.scalar.activation(sqd[:], ln_sb[:], Square,
                                     accum_out=ssq[:])

                if pending_ln:
                    pending_ln.pop(0)()

                def fin_ln(m=m, ms=ms, ln_sb=ln_sb, s1=s1, s2=s2, ssq=ssq):
                    mean = lnp.tile([128, 1], f32, name="mean", tag="mean")
                    nc.vector.tensor_scalar(
                        out=mean[:], in0=s1[:], scalar1=s2[:],
                        scalar2=1.0 / D, op0=Alu.add, op1=Alu.mult,
                    )
                    msq = lnp.tile([128, 1], f32, name="msq", tag="msq")
                    nc.vector.tensor_mul(msq[:], mean[:], mean[:])
                    vt = lnp.tile([128, 1], f32, name="vt", tag="vt")
                    nc.vector.tensor_scalar(
                        out=vt[:], in0=ssq[:], scalar1=1.0 / D,
                        scalar2=msq[:], op0=Alu.mult, op1=Alu.subtract,
                    )
                    stdt = lnp.tile([128, 1], f32, name="stdt", tag="stdt")
                    nc.scalar.activation(stdt[:], vt[:], Sqrt, bias=eps_t[:])
                    rstd = lnp.tile([128, 1], f32, name="rstd", tag="rstd")
                    nc.vector.reciprocal(rstd[:], stdt[:])
                    mrs = lnp.tile([128, 1], f32, name="mrs", tag="mrs")
                    nc.vector.tensor_mul(mrs[:], mean[:], rstd[:])
                    out_sb = lnp.tile([128, D], f32, name="out_sb",
                                      tag="out_sb")
                    nc.vector.tensor_scalar(
                        out=out_sb[:], in0=ln_sb[:],
                        scalar1=rstd[:], scalar2=mrs[:],
                        op0=Alu.mult, op1=Alu.subtract,
                    )
                    nc.sync.dma_start(out=out_d[ms, :], in_=out_sb[:])

                pending_ln.append(fin_ln)

            for fin_ln in pending_ln:
                fin_ln()

    with tile.TileContext(nc) as tc:
        if loop_n is None:
            with tc.sbuf_pool(name="pers", bufs=1) as pers:
                emit(tc, pers)
        else:
            with tc.For_i(0, loop_n, 1):
                for _ in range(unroll):
                    with tc.sbuf_pool(name="pers", bufs=1) as pers:
                        emit(tc, pers)

    nc.compile()
    return nc


def get_program(loop_n=None, unroll=1):
    key = (loop_n, unroll)
    if key not in _PROGRAMS:
        _PROGRAMS[key] = _build_program(loop_n, unroll)
    return _PROGRAMS[key]


def _pack_w(W):
    # [768, 768] -> SBUF layout [128, 6, 768]: (p, t, n) <- W[128t+p, n]
    return np.ascontiguousarray(
        W.astype(bfloat16).reshape(6, 128, D).transpose(1, 0, 2))


def _pack_xT(x):
    # [rows, 768] -> X^T in SBUF layout [128, 6, rows]:
    # (p, t, n) <- X^T[128t+p, n] = x[n, 128t+p]
    return np.ascontiguousarray(
        x.astype(bfloat16).reshape(-1, 6, 128).transpose(2, 1, 0))


def make_input_maps(input_Q, input_K, input_V, Wq, Wk, Wv, Wo):
    wq_bf = _pack_w(Wq)
    wk_bf = _pack_w(Wk)
    wv_bf = _pack_w(Wv)
    wo = _pack_w(Wo)
    # per-batch host-side transposes (shared across the 4 cores of a batch)
    xq_b = [input_Q[b].astype(bfloat16) for b in range(B)]
    xkT_b = [_pack_xT(input_K[b]) for b in range(B)]
    xvT_b = [_pack_xT(input_V[b]) for b in range(B)]
    in_maps = []
    for c in range(NCORES):
        b, r = c // 4, c % 4
        rows = slice(QR * r, QR * (r + 1))
        xq = xq_b[b][rows]
        in_maps.append({
            # residual rows in SBUF layout [128, 4, 768]: (p, m, n) <-
            # xq[128m+p, n]
            "xq_bf": np.ascontiguousarray(
                xq.reshape(4, 128, D).transpose(1, 0, 2)),
            "xqT_bf": _pack_xT(xq),
            "xkT_bf": xkT_b[b],
            "xvT_bf": xvT_b[b],
            "wq_bf": wq_bf,
            "wk_bf": wk_bf,
            "wv_bf": wv_bf,
            "wo": wo,
        })
    return in_maps


def kernel(input_Q, input_K, input_V, Wq, Wk, Wv, Wo):
    from concourse.bass_utils import run_bass_kernel_spmd

    nc = get_program()
    in_maps = make_input_maps(input_Q, input_K, input_V, Wq, Wk, Wv, Wo)
    res = run_bass_kernel_spmd(nc, in_maps, list(range(NCORES)))
    out = np.empty((B, S, D), np.float32)
    for c in range(NCORES):
        b, r = c // 4, c % 4
        out[b, QR * r:QR * (r + 1)] = res.results[c]["out"]
    return out
